# revision 1
# baseline (speedup 1.0000x reference)
"""BiLSTM Trainium2 kernel.

Problem: B=64, T=512, D=U=512. Two independent LSTMs (fwd on xf, bwd on xb),
outputs concatenated on the feature dim.

Sharding: direction-split x batch-split. Cores 0-3 run the forward LSTM
(16 batch rows each), cores 4-7 the backward LSTM. No collectives; the
per-core weights/inputs differ only through the input maps (same SPMD
program on all 8 cores).

Per core:
  Phase 1 (proj): xz = x @ W + b as a dense matmul over all T*B_loc rows
    (bias folded in via a K=1 ones-row matmul), written to a DRAM scratch.
  Phase 2 (recurrence): for t in range(T):
      z = h_{t-1} @ U + xz_t   (float32r matmuls, xz_t injected into the
                                same PSUM accumulation via an identity
                                stationary matmul)
      i,f,o = sigmoid(z[...]), g = tanh(z[...])   (gate columns pre-permuted
                                to [i|f|o|g] per 256-col chunk so one ACT
                                instruction covers i,f,o)
      c = f*c + i*g ; h = o*tanh(c)
      h transposed back to [U, B] layout via PE-transpose for the next
      step's stationary operand. Two 256-column chunks per step pipeline
      the ACT/DVE tail under the PE streaming of the next chunk/step.
"""

import os
import sys

sys.path.insert(0, "/opt/trn_rl_repo")

import numpy as np
import ml_dtypes
from contextlib import ExitStack

import concourse.bass as bass  # noqa: F401
import concourse.tile as tile
from concourse import bacc, mybir
from concourse.bass_utils import run_bass_kernel_spmd

B, T, D, U = 64, 512, 512, 512
G = 4 * U                      # gate width 2048
NCORE = 8
NDIR_CORES = 4                 # cores per direction
B_LOC = B // NDIR_CORES        # 16
NCHUNK = int(os.environ.get("BK_NCHUNK", "4"))  # h-column chunks per step
CH = U // NCHUNK
LAST_FIRST = int(os.environ.get("BK_LAST_FIRST", "0"))  # finish last chunk's bank first
SPLIT_SIG = int(os.environ.get("BK_SPLIT_SIG", "0"))    # separate if-sig and o-sig
TG_FIRST = int(os.environ.get("BK_TG_FIRST", "0"))      # emit tanh_g before sigmoid
GP_FC = int(os.environ.get("BK_GP_FC", "1"))            # fc on gpsimd
SIGALL = int(os.environ.get("BK_SIGALL", "1"))          # one sigmoid for all 4 gates (g pre-scaled x2)
GP_AFF = int(os.environ.get("BK_GP_AFF", "0"))          # g-affine fixup on gpsimd
ZBUFS = int(os.environ.get("BK_ZBUFS", "3"))            # z psum bank buffers
GBUFS = int(os.environ.get("BK_GBUFS", "3"))            # gates pool buffers
SBUFS = int(os.environ.get("BK_SBUFS", "2"))            # state pool buffers
PREM = int(os.environ.get("BK_PREM", "4"))              # proj m-tiles emitted before the loop
PJBUFS = int(os.environ.get("BK_PJBUFS", "2"))          # proj psum buffers
ALTCP = int(os.environ.get("BK_ALTCP", "0"))            # alternate hT-copy engine ACT/DVE
XBUFS = int(os.environ.get("BK_XBUFS", "3"))            # xz_t pool buffers
HBUFS = int(os.environ.get("BK_HBUFS", "3"))            # hT-transpose psum buffers
PAIR = int(os.environ.get("BK_PAIR", "0"))              # single z psum tile; chains at 256-col pair granularity
FPOS = int(os.environ.get("BK_FPOS", "0"))              # proj filler position: 0=after chains, 1=after round 2
DMAT = int(os.environ.get("BK_DMAT", "0"))              # h transpose via DMA xbar (bf16 hT)

F32 = mybir.dt.float32
F32R = mybir.dt.float32r
BF16 = mybir.dt.bfloat16
AF = mybir.ActivationFunctionType


def _gate_perm():
    """New gate-column order: per 256-chunk c: [i_c, f_c, o_c, g_c].

    Original Keras order along 4U: [i(0:U), f(U:2U), g(2U:3U), o(3U:4U)].
    """
    idx = []
    for c in range(NCHUNK):
        s = c * CH
        for g0 in (0, U, 3 * U, 2 * U):  # i, f, o, g
            idx.append(np.arange(g0 + s, g0 + s + CH))
    return np.concatenate(idx)


def _emit(tc, nc, xT, Wp, Up, eye, eyer, onesr, zerosr, zerosb, hs, t_steps, b_loc):
    rt = t_steps * b_loc
    n_m = rt // 128

    with ExitStack() as es:
        consts = es.enter_context(tc.tile_pool(name="consts", bufs=1))
        dramp = es.enter_context(tc.tile_pool(name="dram", bufs=1, space="DRAM"))

        xz = dramp.tile([rt, G], F32R, tag="xz")

        w_t = consts.tile([128, 4, G], F32R, tag="w")
        u_t = consts.tile([128, 4, G], F32R, tag="u")
        for k in range(4):
            nc.sync.dma_start(out=w_t[:, k, :], in_=Wp[128 * k:128 * (k + 1), :])
            nc.sync.dma_start(out=u_t[:, k, :], in_=Up[128 * k:128 * (k + 1), :])
        wb_t = consts.tile([1, G], F32R, tag="wb")
        nc.sync.dma_start(out=wb_t, in_=Wp[D:D + 1, :])
        eye_t = consts.tile([b_loc, b_loc], F32, tag="eye")
        nc.sync.dma_start(out=eye_t, in_=eye)
        eyer_t = consts.tile([b_loc, b_loc], F32R, tag="eyer")
        nc.sync.dma_start(out=eyer_t, in_=eyer)
        ones_t = consts.tile([1, 128], F32R, tag="ones")
        nc.sync.dma_start(out=ones_t, in_=onesr)

        # ---- xz = x @ W + b : emission helper --------------------------
        # The first PRE m-tiles are emitted before the recurrence loop; the
        # rest are interleaved one-per-8-steps into the loop so their matmuls
        # fill the PE bubbles while it waits on the gate chain, and xz stays
        # ~128 steps ahead of consumption.
        es2 = es.enter_context(ExitStack())
        xkp = es2.enter_context(tc.tile_pool(name="xk", bufs=3))
        pcp = es2.enter_context(tc.tile_pool(name="pc", bufs=3))
        pjps = es2.enter_context(tc.tile_pool(name="pjps", bufs=PJBUFS,
                                              space="PSUM"))

        xk_tiles = {}

        def emit_proj_n(m, n):
            if n == 0:
                xk = xkp.tile([128, 4, 128], F32R, tag="xk", name=f"xk_{m}")
                nc.sync.dma_start(
                    out=xk,
                    in_=xT[:, 128 * m:128 * (m + 1)].rearrange(
                        "(k p) m -> p k m", p=128),
                )
                xk_tiles[m] = xk
            xk = xk_tiles[m]
            ps = pjps.tile([128, 512], F32, tag="pj", name=f"pj_{m}_{n}")
            for k in range(4):
                nc.tensor.matmul(
                    ps, xk[:, k, :], w_t[:, k, 512 * n:512 * (n + 1)],
                    start=(k == 0), stop=False)
            nc.tensor.matmul(
                ps, ones_t, wb_t[:, 512 * n:512 * (n + 1)],
                start=False, stop=True)
            oc = pcp.tile([128, 512], F32R, tag="oc", name=f"oc_{m}_{n}")
            nc.vector.tensor_copy(oc, ps)
            nc.sync.dma_start(
                out=xz[128 * m:128 * (m + 1), 512 * n:512 * (n + 1)],
                in_=oc)

        def emit_proj(m):
            for n in range(4):
                emit_proj_n(m, n)

        PRE = min(n_m, PREM)
        for m in range(PRE):
            emit_proj(m)
        next_m = PRE * 4   # counted in quarters now

        # ---- Phase 2: recurrence ----------------------------------------
        with tc.tile_pool(name="xzp", bufs=XBUFS) as xzp, \
             tc.tile_pool(name="state", bufs=SBUFS) as state, \
             tc.tile_pool(name="gates", bufs=GBUFS) as gp, \
             tc.tile_pool(name="zps",
                          bufs=(1 if PAIR else ZBUFS),
                          space="PSUM") as zps, \
             tc.tile_pool(name="hps", bufs=HBUFS, space="PSUM") as hps:

            n_state = NCHUNK // 2 if PAIR else NCHUNK
            kslices = 4 // n_state
            hT_prev = [state.tile([128, kslices * b_loc],
                                  BF16 if DMAT else F32R,
                                  tag=f"hT{c}", name=f"hT_init{c}")
                       for c in range(n_state)]
            c_prev = [state.tile([b_loc, (4 // n_state) * CH], F32,
                                 tag=(f"cp{c}" if PAIR else f"c{c}"),
                                 name=f"c_init{c}")
                      for c in range(n_state)]
            for c in range(n_state):
                zsrc = (zerosb if DMAT else zerosr)
                nc.sync.dma_start(out=hT_prev[c],
                                  in_=zsrc[:, :kslices * b_loc])
                nc.vector.memset(c_prev[c][:], 0.0)

            for t in range(t_steps):
                xz_t = xzp.tile([b_loc, G], F32R, tag="xz_t")
                nc.sync.dma_start(out=xz_t, in_=xz[b_loc * t:b_loc * (t + 1), :])
                # chunk c covers z columns [c*4*CH, (c+1)*4*CH) and h/c
                # columns [c*CH, (c+1)*CH); KPC k-tiles per chunk.
                KPC = 4 // NCHUNK
                CW = 4 * CH                 # z-columns per chunk
                JPC = CW // 512             # 512-wide matmul slices per chunk
                if PAIR:
                    zbig = zps.tile([b_loc, G], F32, tag="z",
                                    name=f"z_{t}")
                    pss = [zbig[:, CW * c:CW * (c + 1)]
                           for c in range(NCHUNK)]
                else:
                    pss = [zps.tile([b_loc, CW], F32, tag="z",
                                    name=f"z_{t}_{c}") for c in range(NCHUNK)]

                def mm(c, j, k, start=False, stop=False):
                    n0 = CW * c + 512 * j
                    pj = pss[c][:, 512 * j:512 * (j + 1)]
                    if k == 4:
                        nc.tensor.matmul(pj, eyer_t, xz_t[:, n0:n0 + 512],
                                         start=start, stop=stop,
                                         skip_group_check=True)
                    else:
                        kpt = 4 // len(hT_prev)
                        src_c, kk = divmod(k, kpt)
                        nc.tensor.matmul(
                            pj,
                            hT_prev[src_c][:, b_loc * kk:b_loc * (kk + 1)],
                            u_t[:, k, n0:n0 + 512],
                            start=start, stop=stop, skip_group_check=True)

                # xz injects first: they have no h dependency, so they fill
                # the PE bubble at step start and each bank then completes at
                # its last k-matmul instead of waiting for a trailing inject.
                for c in range(NCHUNK):
                    for j in range(JPC):
                        mm(c, j, 4, start=True)
                # k-rounds ordered by which hT chunk they need, so the PE can
                # start as soon as the earliest chunk of hT(t-1) lands.
                for r in range(NCHUNK - 1):
                    for c in range(NCHUNK):
                        for j in range(JPC):
                            for k in range(r * KPC, (r + 1) * KPC):
                                mm(c, j, k)
                if FPOS == 1 and t % 2 == 0 and next_m < 4 * n_m:
                    emit_proj_n(next_m // 4, next_m % 4)
                    next_m += 1
                hT_new, c_new = [None] * NCHUNK, [None] * NCHUNK
                hns = [None] * NCHUNK
                corder = (list(range(NCHUNK - 1, -1, -1)) if LAST_FIRST
                          else list(range(NCHUNK)))
                if PAIR:
                    for pr in range(NCHUNK // 2):
                        c0, c1 = 2 * pr, 2 * pr + 1
                        for c in (c0, c1):
                            for j in range(JPC):
                                for k in range((NCHUNK - 1) * KPC,
                                               NCHUNK * KPC):
                                    mm(c, j, k,
                                       stop=(k == NCHUNK * KPC - 1))
                        W2 = 2 * CH
                        ps2 = zbig[:, CW * c0:CW * (c1 + 1)]
                        sig = gp.tile([b_loc, 2 * CW], F32, tag=f"sigp{pr}",
                                      name=f"sig_{t}_{pr}")
                        nc.scalar.activation(sig, ps2, AF.Sigmoid)
                        tg = gp.tile([b_loc, W2], F32, tag=f"tgp{pr}",
                                     name=f"tg_{t}_{pr}")
                        aff_eng = nc.gpsimd if GP_AFF else nc.vector
                        for ci in range(2):
                            aff_eng.tensor_scalar(
                                tg[:, CH * ci:CH * (ci + 1)],
                                sig[:, CW * ci + 3 * CH:CW * ci + 4 * CH],
                                2.0, 1.0, mybir.AluOpType.mult,
                                mybir.AluOpType.subtract)
                        fc = gp.tile([b_loc, W2], F32, tag=f"fcp{pr}",
                                     name=f"fc_{t}_{pr}")
                        fc_eng = nc.gpsimd if GP_FC else nc.vector
                        cpv = c_prev[pr]
                        for ci in range(2):
                            fc_eng.tensor_mul(
                                fc[:, CH * ci:CH * (ci + 1)],
                                sig[:, CW * ci + CH:CW * ci + 2 * CH],
                                cpv[:, CH * ci:CH * (ci + 1)])
                        ig = gp.tile([b_loc, W2], F32, tag=f"igp{pr}",
                                     name=f"ig_{t}_{pr}")
                        for ci in range(2):
                            nc.vector.tensor_mul(
                                ig[:, CH * ci:CH * (ci + 1)],
                                sig[:, CW * ci:CW * ci + CH],
                                tg[:, CH * ci:CH * (ci + 1)])
                        cn = state.tile([b_loc, W2], F32, tag=f"cp{pr}",
                                        name=f"c_{t}_{pr}")
                        nc.vector.tensor_add(cn, ig, fc)
                        tch = gp.tile([b_loc, W2], F32, tag=f"tcp{pr}",
                                      name=f"tc_{t}_{pr}")
                        nc.scalar.activation(tch, cn, AF.Tanh)
                        hn = gp.tile([b_loc, W2], F32, tag=f"hp{pr}",
                                     name=f"h_{t}_{pr}")
                        for ci in range(2):
                            nc.vector.tensor_mul(
                                hn[:, CH * ci:CH * (ci + 1)],
                                sig[:, CW * ci + 2 * CH:CW * ci + 3 * CH],
                                tch[:, CH * ci:CH * (ci + 1)])
                        nc.sync.dma_start(
                            out=hs[t, :, W2 * pr:W2 * (pr + 1)], in_=hn)
                        c_new[pr] = cn
                        hns[pr] = hn

                    if FPOS == 0 and t % 2 == 0 and next_m < 4 * n_m:
                        emit_proj_n(next_m // 4, next_m % 4)
                        next_m += 1

                    for pr in range(NCHUNK // 2):
                        hn = hns[pr]
                        hT = state.tile([128, 2 * b_loc], F32R,
                                        tag=f"hT{pr}", name=f"hT_{t}_{pr}")
                        for kk in range(2):
                            pt = hps.tile([128, b_loc], F32, tag="pt",
                                          name=f"pt_{t}_{pr}_{kk}")
                            nc.tensor.transpose(
                                pt, hn[:, CH * kk:CH * (kk + 1)], eye_t)
                            nc.vector.tensor_copy(
                                hT[:, b_loc * kk:b_loc * (kk + 1)], pt)
                        hT_new[pr] = hT
                    hT_prev, c_prev = hT_new[:2], c_new[:2]
                    continue
                for c in corder:
                    for j in range(JPC):
                        for k in range((NCHUNK - 1) * KPC, NCHUNK * KPC):
                            mm(c, j, k, stop=(k == NCHUNK * KPC - 1))
                    ps = pss[c]
                    if SIGALL:
                        # g columns were pre-scaled x2 host-side;
                        # tanh(x) = 2*sigmoid(2x) - 1 makes one sigmoid
                        # instruction cover all four gates of the chunk.
                        sig = gp.tile([b_loc, 4 * CH], F32, tag=f"sig{c}",
                                      name=f"sig_{t}_{c}")
                        nc.scalar.activation(sig, ps[:, 0:4 * CH], AF.Sigmoid)
                        tg = gp.tile([b_loc, CH], F32, tag=f"tg{c}",
                                     name=f"tg_{t}_{c}")
                        aff_eng = nc.gpsimd if GP_AFF else nc.vector
                        aff_eng.tensor_scalar(
                            tg, sig[:, 3 * CH:4 * CH], 2.0, 1.0,
                            mybir.AluOpType.mult, mybir.AluOpType.subtract)
                    else:
                        sig = gp.tile([b_loc, 3 * CH], F32, tag=f"sig{c}",
                                      name=f"sig_{t}_{c}")
                        nc.scalar.activation(sig, ps[:, 0:3 * CH], AF.Sigmoid)
                        tg = gp.tile([b_loc, CH], F32, tag=f"tg{c}",
                                     name=f"tg_{t}_{c}")
                        nc.scalar.activation(tg, ps[:, 3 * CH:4 * CH], AF.Tanh)
                    fc = gp.tile([b_loc, CH], F32, tag=f"fc{c}",
                                 name=f"fc_{t}_{c}")
                    fc_eng = nc.gpsimd if GP_FC else nc.vector
                    fc_eng.tensor_mul(fc, sig[:, CH:2 * CH], c_prev[c])
                    ig = gp.tile([b_loc, CH], F32, tag=f"ig{c}",
                                 name=f"ig_{t}_{c}")
                    nc.vector.tensor_mul(ig, sig[:, 0:CH], tg)
                    cn = state.tile([b_loc, CH], F32, tag=f"c{c}", name=f"c_{t}_{c}")
                    nc.vector.tensor_add(cn, ig, fc)
                    tch = gp.tile([b_loc, CH], F32, tag=f"tc{c}",
                                  name=f"tc_{t}_{c}")
                    nc.scalar.activation(tch, cn, AF.Tanh)
                    hn = gp.tile([b_loc, CH], F32, tag=f"h{c}",
                                 name=f"h_{t}_{c}")
                    nc.vector.tensor_mul(hn, sig[:, 2 * CH:3 * CH], tch)
                    nc.sync.dma_start(
                        out=hs[t, :, CH * c:CH * (c + 1)], in_=hn)
                    hns[c] = hn
                    c_new[c] = cn

                # proj work lands here in PE program order: it fills the
                # bubble while the PE waits for the first gate chain.
                if t % 2 == 0 and next_m < 4 * n_m:
                    emit_proj_n(next_m // 4, next_m % 4)
                    next_m += 1

                for c in corder:
                    hn = hns[c]
                    hT = state.tile([128, KPC * b_loc], F32R,
                                    tag=f"hT{c}", name=f"hT_{t}_{c}")
                    for kk in range(KPC):
                        pt = hps.tile([128, b_loc], F32, tag="pt",
                                      name=f"pt_{t}_{c}_{kk}")
                        nc.tensor.transpose(
                            pt, hn[:, 128 * kk:128 * (kk + 1)], eye_t)
                        if ALTCP and c % 2 == 1:
                            nc.scalar.copy(
                                hT[:, b_loc * kk:b_loc * (kk + 1)], pt)
                        else:
                            nc.vector.tensor_copy(
                                hT[:, b_loc * kk:b_loc * (kk + 1)], pt)
                    hT_new[c] = hT
                hT_prev, c_prev = hT_new, c_new


def build_program(t_steps=T, b_loc=B_LOC):
    rt = t_steps * b_loc
    nc = bacc.Bacc("TRN2", target_bir_lowering=False, debug=False,
                   num_devices=NCORE)
    xT = nc.dram_tensor("xT", [D, rt], F32R, kind="ExternalInput").ap()
    Wp = nc.dram_tensor("Wp", [D + 1, G], F32R, kind="ExternalInput").ap()
    Up = nc.dram_tensor("Up", [U, G], F32R, kind="ExternalInput").ap()
    eye = nc.dram_tensor("eye", [b_loc, b_loc], F32, kind="ExternalInput").ap()
    eyer = nc.dram_tensor("eyer", [b_loc, b_loc], F32R,
                          kind="ExternalInput").ap()
    onesr = nc.dram_tensor("onesr", [1, 128], F32R, kind="ExternalInput").ap()
    zerosr = nc.dram_tensor("zerosr", [128, 2 * b_loc], F32R,
                            kind="ExternalInput").ap()
    zerosb = nc.dram_tensor("zerosb", [128, 2 * b_loc], mybir.dt.bfloat16,
                            kind="ExternalInput").ap()
    hs = nc.dram_tensor("hs", [t_steps, b_loc, U], F32,
                        kind="ExternalOutput").ap()
    with tile.TileContext(nc) as tc:
        _emit(tc, nc, xT, Wp, Up, eye, eyer, onesr, zerosr, zerosb, hs, t_steps, b_loc)
    nc.compile()
    return nc


_CACHE = {}


def _get_program(t_steps=T, b_loc=B_LOC):
    key = (t_steps, b_loc)
    if key not in _CACHE:
        _CACHE[key] = build_program(t_steps, b_loc)
    return _CACHE[key]


def make_in_maps(xf, xb, Wf, Uf, bf, Wb, Ub, bb, t_steps=T, b_loc=B_LOC):
    perm = _gate_perm()
    eye = np.eye(b_loc, dtype=np.float32)
    packs = {}
    gscale = np.ones(G, np.float32)
    if SIGALL:
        for c in range(NCHUNK):
            gscale[(4 * c + 3) * CH:(4 * c + 4) * CH] = 2.0
    for d, (W, Urec, bias) in enumerate(((Wf, Uf, bf), (Wb, Ub, bb))):
        Wp = np.ascontiguousarray(
            np.concatenate([W, bias[None, :]], axis=0)[:, perm] * gscale)
        Upp = np.ascontiguousarray(Urec[:, perm] * gscale)
        packs[d] = (Wp, Upp)
    in_maps = []
    for core in range(NCORE):
        d, j = divmod(core, NDIR_CORES)
        x = (xf if d == 0 else xb)[b_loc * j:b_loc * (j + 1), :t_steps]
        # xT[d, t*b_loc + b] = x[b, t, d]
        xT = np.ascontiguousarray(
            x.transpose(2, 1, 0).reshape(D, t_steps * b_loc))
        Wp, Upp = packs[d]
        in_maps.append({"xT": xT, "Wp": Wp, "Up": Upp, "eye": eye,
                        "eyer": eye,
                        "onesr": np.ones((1, 128), np.float32),
                        "zerosr": np.zeros((128, 2 * b_loc), np.float32),
                        "zerosb": np.zeros((128, 2 * b_loc),
                                           ml_dtypes.bfloat16)})
    return in_maps


def kernel(xf, xb, Wf, Uf, bf, Wb, Ub, bb):
    xf = np.asarray(xf, np.float32)
    xb = np.asarray(xb, np.float32)
    Wf = np.asarray(Wf, np.float32)
    Uf = np.asarray(Uf, np.float32)
    bf = np.asarray(bf, np.float32)
    Wb = np.asarray(Wb, np.float32)
    Ub = np.asarray(Ub, np.float32)
    bb = np.asarray(bb, np.float32)

    nc = _get_program()
    in_maps = make_in_maps(xf, xb, Wf, Uf, bf, Wb, Ub, bb)
    res = run_bass_kernel_spmd(nc, in_maps, list(range(NCORE)))

    out = np.empty((B, T, 2 * U), np.float32)
    for core in range(NCORE):
        d, j = divmod(core, NDIR_CORES)
        hsv = res.results[core]["hs"]  # [T, b_loc, U]
        out[B_LOC * j:B_LOC * (j + 1), :, U * d:U * (d + 1)] = \
            hsv.transpose(1, 0, 2)
    return out



# revision 9
# speedup vs baseline: 2.7584x; 2.7584x over previous
"""BiLSTM Trainium2 kernel — transposed/fused formulation (V3).

Problem: B=64, T=512, D=U=512. Two independent LSTMs (fwd on xf, bwd on xb),
outputs concatenated on the feature dim.

Sharding: direction-split x batch-split. Cores 0-3 run the forward LSTM
(16 batch rows each), cores 4-7 the backward LSTM. No collectives.

Formulation (everything transposed — batch is the PE moving/free dim):
  z^T[gate_col, b] = U^T h^T + W^T x_t^T + bias, computed as 128-col gate
  tiles (16 tiles x 4 k-chunks) of tiny fp16 matmuls with the WEIGHTS
  stationary and h^T/x^T ([128, 16]) streaming.  The W-part + bias for step
  t+1 are emitted right after the U-part of step t: no recurrence dep, so
  they fill the PE while the gate chain runs.

  Gate tiles are ordered [f|g|i|o] and z is accumulated in three separate
  PSUM tiles {f}, {g,i}, {o} so sigmoid(f) can issue after only 16 of the 64
  U-matmuls, overlapping ACT with the rest of the PE stream.  g columns are
  pre-scaled x2 host-side (tanh(x) = 2*sigmoid(2x) - 1).

  Cell update per step (custom DVE ops registered at import time; the DVE
  micro-op table is per-NEFF, no firmware change):
    fc  = sig_f * c                      (Pool, stock)
    ig  = sig_i * (2*sig_g - 1)          (BK_IG_TANH, fused)
    c'  = clamp(ig + fc, +-2.6)          (BK_CLAMP_ADD; real |c| <= 2.45 so
                                          the clamp is inert safety for the
                                          tanh polynomial below)
    r   = c'*(a0 + a1 y + a2 y^2)        (BK_TANH_A, y = c'^2)
    T   = r + c' y^3 (a3 + a4 y)         (BK_TANH_B; deg-9 odd minimax of
                                          tanh on [0, 2.6], max err 2.7e-3)
    h   = sig_o * T  (fp16)              (DVE stock mul)
  h is written into the output staging tile, which is also the next step's
  matmul rhs — no transposes anywhere.  x streams in as fp16 [128,(t,k,b)];
  h streams out as fp16 [128,(t,c,b)] every OB steps.
"""

import os
import sys

sys.path.insert(0, "/opt/trn_rl_repo")

import numpy as np
from contextlib import ExitStack

import concourse.bass as bass  # noqa: F401
import concourse.tile as tile
from concourse import bacc, mybir
from concourse.bass_utils import run_bass_kernel_spmd

B, T, D, U = 64, 512, 512, 512
G = 4 * U                      # gate width 2048
NCORE = 8
NDIR_CORES = 4                 # cores per direction
B_LOC = B // NDIR_CORES        # 16
NT = 16                        # gate tiles (G / 128)
NK = 4                         # k chunks (D / 128)
CB = NK * B_LOC                # cell free width (4 chunks x 16 batch) = 64

XB = int(os.environ.get("BK_XB", "8"))      # steps per x DMA block
OB = int(os.environ.get("BK_OB", "8"))      # steps per output DMA block
NDUM = int(os.environ.get("BK_NDUM", "0"))  # dummy filler matmuls per step
ZBUFS = int(os.environ.get("BK_ZBUFS", "2"))
POOL_FC = int(os.environ.get("BK_POOL_FC", "0"))  # fc on gpsimd
POOL_H = int(os.environ.get("BK_POOL_H", "0"))    # h-mul on gpsimd
ACT_TANH = int(os.environ.get("BK_ACT_TANH", "0"))  # cell tanh on ACT instead of poly

CLAMP_C = 2.6
# deg-9 odd minimax coeffs for tanh on [0, 2.6] (max abs err 2.7e-3)
TA = (0.9866325884863426, -0.278550831175462, 0.0637625184246867)
TB = (-0.008001787662182125, 0.00040027875656558184)

F32 = mybir.dt.float32
F16 = mybir.dt.float16
AF = mybir.ActivationFunctionType
ALU = mybir.AluOpType

_BK_OPS = None


def _register_custom_ops():
    """Register our fused DVE ops in dve_ops.OPS (idempotent)."""
    global _BK_OPS
    if _BK_OPS is not None:
        return _BK_OPS
    import concourse.dve_ops as DO
    from concourse.dve_spec import (Spec, Src0, Src1, C0, C1, C2, One,
                                    lower, minn, maxx, sq)
    from concourse.dve_uop import DveOpSpec

    have = {op.name: op for op in DO.OPS if op.name.startswith("BK_")}
    if have:
        _BK_OPS = have
        return have

    y = sq(Src0)
    y2 = y * y
    specs = {
        # ig = si * (2*sg - 1)
        "BK_IG_TANH": Spec(
            body=Src0 * (Src1 + Src1 - One),
            reference=lambda in0, in1, s0, s1, imm2: in0 * (2.0 * in1 - 1.0)),
        # c' = clamp(ig + fc, s0, s1)
        "BK_CLAMP_ADD": Spec(
            body=minn(maxx(Src0 + Src1, C0), C1),
            reference=lambda in0, in1, s0, s1, imm2: np.clip(
                in0 + in1, s0, s1)),
        # r = x * (C0 + C1 y + C2 y^2)
        "BK_TANH_A": Spec(
            body=((C0 + C1 * y) + C2 * y2) * Src0,
            reference=lambda in0, s0, s1, imm2: in0 * (
                s0 + s1 * in0 * in0 + imm2 * (in0 * in0) ** 2)),
        # T = r + (x * y^3) * (C0 + C1 y)
        "BK_TANH_B": Spec(
            body=Src1 + (Src0 * (y * y2)) * (C0 + C1 * y),
            reference=lambda in0, in1, s0, s1, imm2: in1 + in0 * (
                in0 * in0) ** 3 * (s0 + s1 * in0 * in0)),
    }
    out = {}
    for name, spec in specs.items():
        row = DO._CUSTOM_DVE_ROW_BASE + len(DO.OPS)
        shas = {}
        for ver in ("v3", "v4"):
            tmp = DveOpSpec(name=name, opcode=row,
                            uops=lower(spec, ver=ver),
                            rd1_en=DO.has_src1(spec))
            shas[ver] = tmp.sha(ver)
        op = DO.DveOp(name, spec, subdim=False, uops_sha=shas)
        DO.OPS.append(op)
        DO.CUSTOM_DVE_SPECS[name] = spec
        DO._SUB_OPCODE_FOR_NAME[name] = row
        out[name] = op
    _BK_OPS = out
    return out


def _gate_perm_scale():
    """Tile order [f|g|i|o] (4 x 128-col tiles per gate); g scaled x2.

    Keras order along 4U: [i(0:U), f(U:2U), g(2U:3U), o(3U:4U)].
    """
    idx = []
    for g0 in (U, 2 * U, 0, 3 * U):  # f, g, i, o
        idx.append(np.arange(g0, g0 + U))
    perm = np.concatenate(idx)
    scale = np.ones(G, np.float32)
    scale[U:2 * U] = 2.0  # g (new position)
    return perm, scale


# z column groups in tile space: f = tiles 0..3, g,i = 4..11, o = 12..15
GRP = ((0, 4), (4, 12), (12, 16))


def _emit(tc, nc, xT, U16, W16, biasT, ind16, hsT, t_steps):
    b = B_LOC
    ops = _register_custom_ops()
    with ExitStack() as es:
        consts = es.enter_context(tc.tile_pool(name="consts", bufs=1))

        u_t = consts.tile([128, NK, NT, 128], F16, tag="u")
        nc.sync.dma_start(out=u_t, in_=U16)
        w_t = consts.tile([128, NK, NT, 128], F16, tag="w")
        nc.sync.dma_start(out=w_t, in_=W16)
        bias_t = consts.tile([NT, 128], F16, tag="bias")
        nc.sync.dma_start(out=bias_t, in_=biasT)
        ind_t = consts.tile([NT, NT * b], F16, tag="ind")
        nc.sync.dma_start(out=ind_t, in_=ind16)
        hz = consts.tile([128, CB], F16, tag="hz")
        nc.vector.memset(hz[:], 0.0)

        xp = es.enter_context(tc.tile_pool(name="xp", bufs=3))
        zfp = es.enter_context(tc.tile_pool(name="zf", bufs=ZBUFS, space="PSUM"))
        zgip = es.enter_context(tc.tile_pool(name="zgi", bufs=ZBUFS, space="PSUM"))
        zop = es.enter_context(tc.tile_pool(name="zo", bufs=ZBUFS, space="PSUM"))
        scrp = (es.enter_context(tc.tile_pool(name="scr", bufs=1,
                                              space="PSUM"))
                if NDUM else None)
        sp = es.enter_context(tc.tile_pool(name="sig", bufs=3))
        gp = es.enter_context(tc.tile_pool(name="gates", bufs=3))
        cp = es.enter_context(tc.tile_pool(name="c", bufs=2))
        op = es.enter_context(tc.tile_pool(name="out", bufs=3))

        scr = scrp.tile([128, 512], F32, tag="scr") if NDUM else None

        n_xblk = (t_steps + XB - 1) // XB
        x_tiles = {}

        def fetch_x(blk):
            if blk >= n_xblk or blk in x_tiles:
                return
            xt = xp.tile([128, XB, NK, b], F16, tag="x", name=f"x_{blk}")
            w = XB * NK * b
            nc.sync.dma_start(out=xt, in_=xT[:, blk * w:(blk + 1) * w])
            x_tiles[blk] = xt

        fetch_x(0)
        fetch_x(1)

        c_prev = cp.tile([128, CB], F32, tag="c", name="c_init")
        nc.vector.memset(c_prev[:], 0.0)
        h_prev = hz

        def new_ztiles(t):
            return (zfp.tile([128, 4 * b], F32, tag="zf", name=f"zf_{t}"),
                    zgip.tile([128, 8 * b], F32, tag="zgi", name=f"zgi_{t}"),
                    zop.tile([128, 4 * b], F32, tag="zo", name=f"zo_{t}"))

        def emit_wpart(zt, t):
            """bias + W x_t into the three psum tiles (start of accum)."""
            blk, off = divmod(t, XB)
            xt = x_tiles[blk]
            for gi, (g0, g1) in enumerate(GRP):
                z = zt[gi]
                nc.tensor.matmul(z, bias_t, ind_t[:, g0 * b:g1 * b],
                                 start=True, stop=False,
                                 skip_group_check=True)
                for c in range(NK):
                    rhs = xt[:, off, c, :]
                    for g in range(g0, g1):
                        nc.tensor.matmul(z[:, (g - g0) * b:(g - g0 + 1) * b],
                                         w_t[:, c, g, :], rhs,
                                         start=False, stop=False,
                                         skip_group_check=True)

        z_cur = new_ztiles(0)
        emit_wpart(z_cur, 0)

        otile = op.tile([128, OB, CB], F16, tag="o", name="ot_0")

        IG = ops["BK_IG_TANH"]
        CLA = ops["BK_CLAMP_ADD"]
        PTA = ops["BK_TANH_A"]
        PTB = ops["BK_TANH_B"]

        for t in range(t_steps):
            # ---- U-part of step t (waits on h(t-1)), group order f, gi, o --
            for gi, (g0, g1) in enumerate(GRP):
                z = z_cur[gi]
                for c in range(NK):
                    rhs = h_prev[:, c * b:(c + 1) * b]
                    for g in range(g0, g1):
                        nc.tensor.matmul(z[:, (g - g0) * b:(g - g0 + 1) * b],
                                         u_t[:, c, g, :], rhs,
                                         start=False, stop=(c == NK - 1),
                                         skip_group_check=True)

            # ---- sigmoids (ACT), group order f, gi, o ----
            sigf = sp.tile([128, 4 * b], F16, tag="sigf", name=f"sigf_{t}")
            nc.scalar.activation(sigf, z_cur[0], AF.Sigmoid)
            siggi = sp.tile([128, 8 * b], F16, tag="siggi", name=f"siggi_{t}")
            nc.scalar.activation(siggi, z_cur[1], AF.Sigmoid)
            sigo = sp.tile([128, 4 * b], F16, tag="sigo", name=f"sigo_{t}")
            nc.scalar.activation(sigo, z_cur[2], AF.Sigmoid)
            sg = siggi[:, 0:CB]
            si = siggi[:, CB:2 * CB]

            # ---- cell chain ----
            fc = gp.tile([128, CB], F32, tag="fc", name=f"fc_{t}")
            fc_eng = nc.gpsimd if POOL_FC else nc.vector
            fc_eng.tensor_mul(fc, sigf, c_prev)
            ig = gp.tile([128, CB], F32, tag="ig", name=f"ig_{t}")
            nc.vector._custom_dve(IG, out=ig, in0=si, in1=sg)
            cn = cp.tile([128, CB], F32, tag="c", name=f"c_{t}")
            tch = gp.tile([128, CB], F16, tag="tch", name=f"tch_{t}")
            if ACT_TANH:
                nc.vector.tensor_add(cn, ig, fc)
                nc.scalar.activation(tch, cn, AF.Tanh)
            else:
                nc.vector._custom_dve(CLA, out=cn, in0=ig, in1=fc,
                                      s0=-CLAMP_C, s1=CLAMP_C)
                pr = gp.tile([128, CB], F32, tag="pr", name=f"pr_{t}")
                nc.vector._custom_dve(PTA, out=pr, in0=cn,
                                      s0=TA[0], s1=TA[1], imm2=TA[2])
                nc.vector._custom_dve(PTB, out=tch, in0=cn, in1=pr,
                                      s0=TB[0], s1=TB[1])
            hsl = otile[:, t % OB, :]
            h_eng = nc.gpsimd if POOL_H else nc.vector
            h_eng.tensor_mul(hsl, sigo, tch)

            h_prev = hsl
            c_prev = cn

            # ---- output DMA every OB steps ----
            if t % OB == OB - 1 or t == t_steps - 1:
                t0 = (t // OB) * OB
                nc.sync.dma_start(
                    out=hsT[:, t0 * CB:(t + 1) * CB],
                    in_=otile[:, 0:(t - t0 + 1), :])
                if t != t_steps - 1:
                    otile = op.tile([128, OB, CB], F16, tag="o",
                                    name=f"ot_{t + 1}")

            # ---- x prefetch ----
            if t % XB == 0:
                fetch_x(t // XB + 2)

            # ---- W-part of step t+1 (PE filler, no recurrence dep) ----
            if t + 1 < t_steps:
                z_next = new_ztiles(t + 1)
                emit_wpart(z_next, t + 1)
                z_cur = z_next

            # ---- dummy PE filler to hold p-state ----
            for dmy in range(NDUM):
                nc.tensor.matmul(scr, u_t[:, 0, 0, :],
                                 u_t[:, dmy % NK, 0:4, :],
                                 start=True, stop=True,
                                 skip_group_check=True)


def build_program(t_steps=T):
    _register_custom_ops()
    nc = bacc.Bacc("TRN2", target_bir_lowering=False, debug=False,
                   num_devices=NCORE)
    xT = nc.dram_tensor("xT", [128, t_steps * NK * B_LOC], F16,
                        kind="ExternalInput").ap()
    U16 = nc.dram_tensor("U16", [128, NK, NT, 128], F16,
                         kind="ExternalInput").ap()
    W16 = nc.dram_tensor("W16", [128, NK, NT, 128], F16,
                         kind="ExternalInput").ap()
    biasT = nc.dram_tensor("biasT", [NT, 128], F16, kind="ExternalInput").ap()
    ind16 = nc.dram_tensor("ind16", [NT, NT * B_LOC], F16,
                           kind="ExternalInput").ap()
    hsT = nc.dram_tensor("hsT", [128, t_steps * CB], F16,
                         kind="ExternalOutput").ap()
    with tile.TileContext(nc) as tc:
        _emit(tc, nc, xT, U16, W16, biasT, ind16, hsT, t_steps)
    nc.compile()
    return nc


_CACHE = {}


def _get_program(t_steps=T):
    key = t_steps
    if key not in _CACHE:
        _CACHE[key] = build_program(t_steps)
    return _CACHE[key]


def make_in_maps(xf, xb, Wf, Uf, bf, Wb, Ub, bb, t_steps=T):
    perm, scale = _gate_perm_scale()
    packs = {}
    for d, (W, Urec, bias) in enumerate(((Wf, Uf, bf), (Wb, Ub, bb))):
        Wp = (W[:, perm] * scale).astype(np.float16)
        Up = (Urec[:, perm] * scale).astype(np.float16)
        bp = (bias[perm] * scale).astype(np.float16)
        # [k-chunk, 128, tile, 128] -> lhsT tiles [128(k), NK, NT, 128(m)]
        U16 = np.ascontiguousarray(
            Up.reshape(NK, 128, NT, 128).transpose(1, 0, 2, 3))
        W16 = np.ascontiguousarray(
            Wp.reshape(NK, 128, NT, 128).transpose(1, 0, 2, 3))
        biasT = np.ascontiguousarray(bp.reshape(NT, 128))
        packs[d] = (U16, W16, biasT)
    ind = np.zeros((NT, NT, B_LOC), np.float16)
    for k in range(NT):
        ind[k, k, :] = 1.0
    ind16 = np.ascontiguousarray(ind.reshape(NT, NT * B_LOC))
    in_maps = []
    for core in range(NCORE):
        d, j = divmod(core, NDIR_CORES)
        x = (xf if d == 0 else xb)[B_LOC * j:B_LOC * (j + 1), :t_steps]
        # xT[p, (t, k, b)] = x[b, t, k*128 + p]
        xT = np.ascontiguousarray(
            x.reshape(B_LOC, t_steps, NK, 128)
             .transpose(3, 1, 2, 0)
             .reshape(128, t_steps * NK * B_LOC)).astype(np.float16)
        U16, W16, biasT = packs[d]
        in_maps.append({"xT": xT, "U16": U16, "W16": W16, "biasT": biasT,
                        "ind16": ind16})
    return in_maps


def kernel(xf, xb, Wf, Uf, bf, Wb, Ub, bb):
    xf = np.asarray(xf, np.float32)
    xb = np.asarray(xb, np.float32)
    Wf = np.asarray(Wf, np.float32)
    Uf = np.asarray(Uf, np.float32)
    bf = np.asarray(bf, np.float32)
    Wb = np.asarray(Wb, np.float32)
    Ub = np.asarray(Ub, np.float32)
    bb = np.asarray(bb, np.float32)

    nc = _get_program()
    in_maps = make_in_maps(xf, xb, Wf, Uf, bf, Wb, Ub, bb)
    res = run_bass_kernel_spmd(nc, in_maps, list(range(NCORE)))

    out = np.empty((B, T, 2 * U), np.float32)
    for core in range(NCORE):
        d, j = divmod(core, NDIR_CORES)
        hsv = np.asarray(res.results[core]["hsT"])  # [128, T*CB] f16
        # hsT[p, (t, c, b)] -> out[b, t, d*512 + c*128 + p]
        hs = hsv.reshape(128, T, NK, B_LOC).transpose(3, 1, 2, 0)
        out[B_LOC * j:B_LOC * (j + 1), :, U * d:U * (d + 1)] = \
            hs.reshape(B_LOC, T, U).astype(np.float32)
    return out


# revision 13
# speedup vs baseline: 2.7780x; 1.0071x over previous
"""BiLSTM Trainium2 kernel — transposed/fused formulation (V3).

Problem: B=64, T=512, D=U=512. Two independent LSTMs (fwd on xf, bwd on xb),
outputs concatenated on the feature dim.

Sharding: direction-split x batch-split. Cores 0-3 run the forward LSTM
(16 batch rows each), cores 4-7 the backward LSTM. No collectives.

Formulation (everything transposed — batch is the PE moving/free dim):
  z^T[gate_col, b] = U^T h^T + W^T x_t^T + bias, computed as 128-col gate
  tiles (16 tiles x 4 k-chunks) of tiny fp16 matmuls with the WEIGHTS
  stationary and h^T/x^T ([128, 16]) streaming.  The W-part + bias for step
  t+1 are emitted right after the U-part of step t: no recurrence dep, so
  they fill the PE while the gate chain runs.

  Gate tiles are ordered [f|g|i|o] and z is accumulated in three separate
  PSUM tiles {f}, {g,i}, {o} so sigmoid(f) can issue after only 16 of the 64
  U-matmuls, overlapping ACT with the rest of the PE stream.  g columns are
  pre-scaled x2 host-side (tanh(x) = 2*sigmoid(2x) - 1).

  Cell update per step (custom DVE ops registered at import time; the DVE
  micro-op table is per-NEFF, no firmware change):
    fc  = sig_f * c                      (Pool, stock)
    ig  = sig_i * (2*sig_g - 1)          (BK_IG_TANH, fused)
    c'  = clamp(ig + fc, +-2.6)          (BK_CLAMP_ADD; real |c| <= 2.45 so
                                          the clamp is inert safety for the
                                          tanh polynomial below)
    r   = c'*(a0 + a1 y + a2 y^2)        (BK_TANH_A, y = c'^2)
    T   = r + c' y^3 (a3 + a4 y)         (BK_TANH_B; deg-9 odd minimax of
                                          tanh on [0, 2.6], max err 2.7e-3)
    h   = sig_o * T  (fp16)              (DVE stock mul)
  h is written into the output staging tile, which is also the next step's
  matmul rhs — no transposes anywhere.  x streams in as fp16 [128,(t,k,b)];
  h streams out as fp16 [128,(t,c,b)] every OB steps.
"""

import os
import sys

sys.path.insert(0, "/opt/trn_rl_repo")

import numpy as np
from contextlib import ExitStack

import concourse.bass as bass  # noqa: F401
import concourse.tile as tile
from concourse import bacc, mybir
from concourse.bass_utils import run_bass_kernel_spmd

B, T, D, U = 64, 512, 512, 512
G = 4 * U                      # gate width 2048
NCORE = 8
NDIR_CORES = 4                 # cores per direction
B_LOC = B // NDIR_CORES        # 16
NT = 16                        # gate tiles (G / 128)
NK = 4                         # k chunks (D / 128)
CB = NK * B_LOC                # cell free width (4 chunks x 16 batch) = 64

XB = int(os.environ.get("BK_XB", "8"))      # steps per x DMA block
OB = int(os.environ.get("BK_OB", "8"))      # steps per output DMA block
NDUM = int(os.environ.get("BK_NDUM", "0"))  # dummy filler matmuls per step
ZBUFS = int(os.environ.get("BK_ZBUFS", "2"))
POOL_FC = int(os.environ.get("BK_POOL_FC", "0"))  # fc on gpsimd
POOL_H = int(os.environ.get("BK_POOL_H", "0"))    # h-mul on gpsimd
ACT_TANH = int(os.environ.get("BK_ACT_TANH", "1"))  # cell tanh on ACT instead of poly
C16 = int(os.environ.get("BK_C16", "0"))            # fp16 cell state (ACT_TANH only)

CLAMP_C = 2.6
# deg-9 odd minimax coeffs for tanh on [0, 2.6] (max abs err 2.7e-3)
TA = (0.9866325884863426, -0.278550831175462, 0.0637625184246867)
TB = (-0.008001787662182125, 0.00040027875656558184)

F32 = mybir.dt.float32
F16 = mybir.dt.float16
AF = mybir.ActivationFunctionType
ALU = mybir.AluOpType

_BK_OPS = None


def _register_custom_ops():
    """Register our fused DVE ops in dve_ops.OPS (idempotent)."""
    global _BK_OPS
    if _BK_OPS is not None:
        return _BK_OPS
    import concourse.dve_ops as DO
    from concourse.dve_spec import (Spec, Src0, Src1, C0, C1, C2, One,
                                    lower, minn, maxx, sq)
    from concourse.dve_uop import DveOpSpec

    have = {op.name: op for op in DO.OPS if op.name.startswith("BK_")}
    if have:
        _BK_OPS = have
        return have

    y = sq(Src0)
    y2 = y * y
    specs = {
        # ig = si * (2*sg - 1)
        "BK_IG_TANH": Spec(
            body=Src0 * (Src1 + Src1 - One),
            reference=lambda in0, in1, s0, s1, imm2: in0 * (2.0 * in1 - 1.0)),
        # c' = clamp(ig + fc, s0, s1)
        "BK_CLAMP_ADD": Spec(
            body=minn(maxx(Src0 + Src1, C0), C1),
            reference=lambda in0, in1, s0, s1, imm2: np.clip(
                in0 + in1, s0, s1)),
        # r = x * (C0 + C1 y + C2 y^2)
        "BK_TANH_A": Spec(
            body=((C0 + C1 * y) + C2 * y2) * Src0,
            reference=lambda in0, s0, s1, imm2: in0 * (
                s0 + s1 * in0 * in0 + imm2 * (in0 * in0) ** 2)),
        # T = r + (x * y^3) * (C0 + C1 y)
        "BK_TANH_B": Spec(
            body=Src1 + (Src0 * (y * y2)) * (C0 + C1 * y),
            reference=lambda in0, in1, s0, s1, imm2: in1 + in0 * (
                in0 * in0) ** 3 * (s0 + s1 * in0 * in0)),
    }
    out = {}
    for name, spec in specs.items():
        row = DO._CUSTOM_DVE_ROW_BASE + len(DO.OPS)
        shas = {}
        for ver in ("v3", "v4"):
            tmp = DveOpSpec(name=name, opcode=row,
                            uops=lower(spec, ver=ver),
                            rd1_en=DO.has_src1(spec))
            shas[ver] = tmp.sha(ver)
        op = DO.DveOp(name, spec, subdim=False, uops_sha=shas)
        DO.OPS.append(op)
        DO.CUSTOM_DVE_SPECS[name] = spec
        DO._SUB_OPCODE_FOR_NAME[name] = row
        out[name] = op
    _BK_OPS = out
    return out


def _gate_perm_scale():
    """Tile order [f|g|i|o] (4 x 128-col tiles per gate); g scaled x2.

    Keras order along 4U: [i(0:U), f(U:2U), g(2U:3U), o(3U:4U)].
    """
    idx = []
    for g0 in (U, 2 * U, 0, 3 * U):  # f, g, i, o
        idx.append(np.arange(g0, g0 + U))
    perm = np.concatenate(idx)
    scale = np.ones(G, np.float32)
    scale[U:2 * U] = 2.0  # g (new position)
    return perm, scale


# z column groups in tile space: f = tiles 0..3, g = 4..7, i = 8..11, o = 12..15
_GRP_OPTS = {
    "f_gi_o": ((0, 4), (4, 12), (12, 16)),
    "f_gio": ((0, 4), (4, 16)),
    "fgi_o": ((0, 12), (12, 16)),
    "f_g_i_o": ((0, 4), (4, 8), (8, 12), (12, 16)),
}
GRP = _GRP_OPTS[os.environ.get("BK_GRP", "f_gi_o")]


def _emit(tc, nc, xT, U16, W16, biasT, ind16, hsT, t_steps):
    b = B_LOC
    ops = _register_custom_ops()
    with ExitStack() as es:
        consts = es.enter_context(tc.tile_pool(name="consts", bufs=1))

        u_t = consts.tile([128, NK, NT, 128], F16, tag="u")
        nc.sync.dma_start(out=u_t, in_=U16)
        w_t = consts.tile([128, NK, NT, 128], F16, tag="w")
        nc.sync.dma_start(out=w_t, in_=W16)
        bias_t = consts.tile([NT, 128], F16, tag="bias")
        nc.sync.dma_start(out=bias_t, in_=biasT)
        ind_t = consts.tile([NT, NT * b], F16, tag="ind")
        nc.sync.dma_start(out=ind_t, in_=ind16)
        hz = consts.tile([128, CB], F16, tag="hz")
        nc.vector.memset(hz[:], 0.0)

        xp = es.enter_context(tc.tile_pool(name="xp", bufs=3))
        zpools = [es.enter_context(tc.tile_pool(name=f"z{gi}", bufs=ZBUFS,
                                                space="PSUM"))
                  for gi in range(len(GRP))]
        scrp = (es.enter_context(tc.tile_pool(name="scr", bufs=1,
                                              space="PSUM"))
                if NDUM else None)
        sp = es.enter_context(tc.tile_pool(name="sig", bufs=3))
        gp = es.enter_context(tc.tile_pool(name="gates", bufs=3))
        cp = es.enter_context(tc.tile_pool(name="c", bufs=2))
        op = es.enter_context(tc.tile_pool(name="out", bufs=3))

        scr = scrp.tile([128, 512], F32, tag="scr") if NDUM else None

        n_xblk = (t_steps + XB - 1) // XB
        x_tiles = {}

        def fetch_x(blk):
            if blk >= n_xblk or blk in x_tiles:
                return
            xt = xp.tile([128, XB, NK, b], F16, tag="x", name=f"x_{blk}")
            w = XB * NK * b
            nc.sync.dma_start(out=xt, in_=xT[:, blk * w:(blk + 1) * w])
            x_tiles[blk] = xt

        fetch_x(0)
        fetch_x(1)

        c_prev = cp.tile([128, CB],
                         F16 if (C16 and ACT_TANH) else F32,
                         tag="c", name="c_init")
        nc.vector.memset(c_prev[:], 0.0)
        h_prev = hz

        def new_ztiles(t):
            return tuple(
                zpools[gi].tile([128, (g1 - g0) * b], F32, tag=f"z{gi}",
                                name=f"z{gi}_{t}")
                for gi, (g0, g1) in enumerate(GRP))

        def emit_wpart(zt, t):
            """bias + W x_t into the three psum tiles (start of accum)."""
            blk, off = divmod(t, XB)
            xt = x_tiles[blk]
            for gi, (g0, g1) in enumerate(GRP):
                z = zt[gi]
                nc.tensor.matmul(z, bias_t, ind_t[:, g0 * b:g1 * b],
                                 start=True, stop=False,
                                 skip_group_check=True)
                for c in range(NK):
                    rhs = xt[:, off, c, :]
                    for g in range(g0, g1):
                        nc.tensor.matmul(z[:, (g - g0) * b:(g - g0 + 1) * b],
                                         w_t[:, c, g, :], rhs,
                                         start=False, stop=False,
                                         skip_group_check=True)

        z_cur = new_ztiles(0)
        emit_wpart(z_cur, 0)

        otile = op.tile([128, OB, CB], F16, tag="o", name="ot_0")

        IG = ops["BK_IG_TANH"]
        CLA = ops["BK_CLAMP_ADD"]
        PTA = ops["BK_TANH_A"]
        PTB = ops["BK_TANH_B"]

        for t in range(t_steps):
            # ---- U-part of step t (waits on h(t-1)), group order f, gi, o --
            for gi, (g0, g1) in enumerate(GRP):
                z = z_cur[gi]
                for c in range(NK):
                    rhs = h_prev[:, c * b:(c + 1) * b]
                    for g in range(g0, g1):
                        nc.tensor.matmul(z[:, (g - g0) * b:(g - g0 + 1) * b],
                                         u_t[:, c, g, :], rhs,
                                         start=False, stop=(c == NK - 1),
                                         skip_group_check=True)

            # ---- sigmoids (ACT), in group order ----
            sig_parts = []
            for gi, (g0, g1) in enumerate(GRP):
                sgt = sp.tile([128, (g1 - g0) * b], F16, tag=f"sig{gi}",
                              name=f"sig{gi}_{t}")
                nc.scalar.activation(sgt, z_cur[gi], AF.Sigmoid)
                sig_parts.append((g0, g1, sgt))

            def gate_slice(t0):
                # gate tiles: f=0..3, g=4..7, i=8..11, o=12..15
                for g0, g1, sgt in sig_parts:
                    if g0 <= t0 < g1:
                        return sgt[:, (t0 - g0) * b:(t0 - g0 + 4) * b]
                raise AssertionError
            sigf = gate_slice(0)
            sg = gate_slice(4)
            si = gate_slice(8)
            sigo = gate_slice(12)

            # ---- cell chain ----
            cdt = F16 if (C16 and ACT_TANH) else F32
            fc = gp.tile([128, CB], cdt, tag="fc", name=f"fc_{t}")
            fc_eng = nc.gpsimd if POOL_FC else nc.vector
            fc_eng.tensor_mul(fc, sigf, c_prev)
            ig = gp.tile([128, CB], cdt, tag="ig", name=f"ig_{t}")
            nc.vector._custom_dve(IG, out=ig, in0=si, in1=sg)
            cn = cp.tile([128, CB], cdt, tag="c", name=f"c_{t}")
            tch = gp.tile([128, CB], F16, tag="tch", name=f"tch_{t}")
            if ACT_TANH:
                nc.vector.tensor_add(cn, ig, fc)
                nc.scalar.activation(tch, cn, AF.Tanh)
            else:
                nc.vector._custom_dve(CLA, out=cn, in0=ig, in1=fc,
                                      s0=-CLAMP_C, s1=CLAMP_C)
                pr = gp.tile([128, CB], F32, tag="pr", name=f"pr_{t}")
                nc.vector._custom_dve(PTA, out=pr, in0=cn,
                                      s0=TA[0], s1=TA[1], imm2=TA[2])
                nc.vector._custom_dve(PTB, out=tch, in0=cn, in1=pr,
                                      s0=TB[0], s1=TB[1])
            hsl = otile[:, t % OB, :]
            h_eng = nc.gpsimd if POOL_H else nc.vector
            h_eng.tensor_mul(hsl, sigo, tch)

            h_prev = hsl
            c_prev = cn

            # ---- output DMA every OB steps ----
            if t % OB == OB - 1 or t == t_steps - 1:
                t0 = (t // OB) * OB
                nc.sync.dma_start(
                    out=hsT[:, t0 * CB:(t + 1) * CB],
                    in_=otile[:, 0:(t - t0 + 1), :])
                if t != t_steps - 1:
                    otile = op.tile([128, OB, CB], F16, tag="o",
                                    name=f"ot_{t + 1}")

            # ---- x prefetch ----
            if t % XB == 0:
                fetch_x(t // XB + 2)

            # ---- W-part of step t+1 (PE filler, no recurrence dep) ----
            if t + 1 < t_steps:
                z_next = new_ztiles(t + 1)
                emit_wpart(z_next, t + 1)
                z_cur = z_next

            # ---- dummy PE filler to hold p-state ----
            for dmy in range(NDUM):
                nc.tensor.matmul(scr, u_t[:, 0, 0, :],
                                 u_t[:, dmy % NK, 0:4, :],
                                 start=True, stop=True,
                                 skip_group_check=True)


def build_program(t_steps=T):
    _register_custom_ops()
    nc = bacc.Bacc("TRN2", target_bir_lowering=False, debug=False,
                   num_devices=NCORE)
    xT = nc.dram_tensor("xT", [128, t_steps * NK * B_LOC], F16,
                        kind="ExternalInput").ap()
    U16 = nc.dram_tensor("U16", [128, NK, NT, 128], F16,
                         kind="ExternalInput").ap()
    W16 = nc.dram_tensor("W16", [128, NK, NT, 128], F16,
                         kind="ExternalInput").ap()
    biasT = nc.dram_tensor("biasT", [NT, 128], F16, kind="ExternalInput").ap()
    ind16 = nc.dram_tensor("ind16", [NT, NT * B_LOC], F16,
                           kind="ExternalInput").ap()
    hsT = nc.dram_tensor("hsT", [128, t_steps * CB], F16,
                         kind="ExternalOutput").ap()
    with tile.TileContext(nc) as tc:
        _emit(tc, nc, xT, U16, W16, biasT, ind16, hsT, t_steps)
    nc.compile()
    return nc


_CACHE = {}


def _get_program(t_steps=T):
    key = t_steps
    if key not in _CACHE:
        _CACHE[key] = build_program(t_steps)
    return _CACHE[key]


def make_in_maps(xf, xb, Wf, Uf, bf, Wb, Ub, bb, t_steps=T):
    perm, scale = _gate_perm_scale()
    packs = {}
    for d, (W, Urec, bias) in enumerate(((Wf, Uf, bf), (Wb, Ub, bb))):
        Wp = (W[:, perm] * scale).astype(np.float16)
        Up = (Urec[:, perm] * scale).astype(np.float16)
        bp = (bias[perm] * scale).astype(np.float16)
        # [k-chunk, 128, tile, 128] -> lhsT tiles [128(k), NK, NT, 128(m)]
        U16 = np.ascontiguousarray(
            Up.reshape(NK, 128, NT, 128).transpose(1, 0, 2, 3))
        W16 = np.ascontiguousarray(
            Wp.reshape(NK, 128, NT, 128).transpose(1, 0, 2, 3))
        biasT = np.ascontiguousarray(bp.reshape(NT, 128))
        packs[d] = (U16, W16, biasT)
    ind = np.zeros((NT, NT, B_LOC), np.float16)
    for k in range(NT):
        ind[k, k, :] = 1.0
    ind16 = np.ascontiguousarray(ind.reshape(NT, NT * B_LOC))
    in_maps = []
    for core in range(NCORE):
        d, j = divmod(core, NDIR_CORES)
        x = (xf if d == 0 else xb)[B_LOC * j:B_LOC * (j + 1), :t_steps]
        # xT[p, (t, k, b)] = x[b, t, k*128 + p]
        xT = np.ascontiguousarray(
            x.reshape(B_LOC, t_steps, NK, 128)
             .transpose(3, 1, 2, 0)
             .reshape(128, t_steps * NK * B_LOC)).astype(np.float16)
        U16, W16, biasT = packs[d]
        in_maps.append({"xT": xT, "U16": U16, "W16": W16, "biasT": biasT,
                        "ind16": ind16})
    return in_maps


def kernel(xf, xb, Wf, Uf, bf, Wb, Ub, bb):
    xf = np.asarray(xf, np.float32)
    xb = np.asarray(xb, np.float32)
    Wf = np.asarray(Wf, np.float32)
    Uf = np.asarray(Uf, np.float32)
    bf = np.asarray(bf, np.float32)
    Wb = np.asarray(Wb, np.float32)
    Ub = np.asarray(Ub, np.float32)
    bb = np.asarray(bb, np.float32)

    nc = _get_program()
    in_maps = make_in_maps(xf, xb, Wf, Uf, bf, Wb, Ub, bb)
    res = run_bass_kernel_spmd(nc, in_maps, list(range(NCORE)))

    out = np.empty((B, T, 2 * U), np.float32)
    for core in range(NCORE):
        d, j = divmod(core, NDIR_CORES)
        hsv = np.asarray(res.results[core]["hsT"])  # [128, T*CB] f16
        # hsT[p, (t, c, b)] -> out[b, t, d*512 + c*128 + p]
        hs = hsv.reshape(128, T, NK, B_LOC).transpose(3, 1, 2, 0)
        out[B_LOC * j:B_LOC * (j + 1), :, U * d:U * (d + 1)] = \
            hs.reshape(B_LOC, T, U).astype(np.float32)
    return out


# revision 14
# speedup vs baseline: 2.8172x; 1.0141x over previous
"""BiLSTM Trainium2 kernel — transposed/fused formulation (V3).

Problem: B=64, T=512, D=U=512. Two independent LSTMs (fwd on xf, bwd on xb),
outputs concatenated on the feature dim.

Sharding: direction-split x batch-split. Cores 0-3 run the forward LSTM
(16 batch rows each), cores 4-7 the backward LSTM. No collectives.

Formulation (everything transposed — batch is the PE moving/free dim):
  z^T[gate_col, b] = U^T h^T + W^T x_t^T + bias, computed as 128-col gate
  tiles (16 tiles x 4 k-chunks) of tiny fp16 matmuls with the WEIGHTS
  stationary and h^T/x^T ([128, 16]) streaming.  The W-part + bias for step
  t+1 are emitted right after the U-part of step t: no recurrence dep, so
  they fill the PE while the gate chain runs.

  Gate tiles are ordered [f|g|i|o] and z is accumulated in three separate
  PSUM tiles {f}, {g,i}, {o} so sigmoid(f) can issue after only 16 of the 64
  U-matmuls, overlapping ACT with the rest of the PE stream.  g columns are
  pre-scaled x2 host-side (tanh(x) = 2*sigmoid(2x) - 1).

  Cell update per step (custom DVE ops registered at import time; the DVE
  micro-op table is per-NEFF, no firmware change):
    fc  = sig_f * c                      (Pool, stock)
    ig  = sig_i * (2*sig_g - 1)          (BK_IG_TANH, fused)
    c'  = clamp(ig + fc, +-2.6)          (BK_CLAMP_ADD; real |c| <= 2.45 so
                                          the clamp is inert safety for the
                                          tanh polynomial below)
    r   = c'*(a0 + a1 y + a2 y^2)        (BK_TANH_A, y = c'^2)
    T   = r + c' y^3 (a3 + a4 y)         (BK_TANH_B; deg-9 odd minimax of
                                          tanh on [0, 2.6], max err 2.7e-3)
    h   = sig_o * T  (fp16)              (DVE stock mul)
  h is written into the output staging tile, which is also the next step's
  matmul rhs — no transposes anywhere.  x streams in as fp16 [128,(t,k,b)];
  h streams out as fp16 [128,(t,c,b)] every OB steps.
"""

import os
import sys

sys.path.insert(0, "/opt/trn_rl_repo")

import numpy as np
from contextlib import ExitStack

import concourse.bass as bass  # noqa: F401
import concourse.tile as tile
from concourse import bacc, mybir
from concourse.bass_utils import run_bass_kernel_spmd

B, T, D, U = 64, 512, 512, 512
G = 4 * U                      # gate width 2048
NCORE = 8
NDIR_CORES = 4                 # cores per direction
B_LOC = B // NDIR_CORES        # 16
NT = 16                        # gate tiles (G / 128)
NK = 4                         # k chunks (D / 128)
CB = NK * B_LOC                # cell free width (4 chunks x 16 batch) = 64

XB = int(os.environ.get("BK_XB", "8"))      # steps per x DMA block
OB = int(os.environ.get("BK_OB", "8"))      # steps per output DMA block
NDUM = int(os.environ.get("BK_NDUM", "0"))  # dummy filler matmuls per step
ZBUFS = int(os.environ.get("BK_ZBUFS", "2"))
POOL_FC = int(os.environ.get("BK_POOL_FC", "0"))  # fc on gpsimd
POOL_H = int(os.environ.get("BK_POOL_H", "0"))    # h-mul on gpsimd
ACT_TANH = int(os.environ.get("BK_ACT_TANH", "1"))  # cell tanh on ACT instead of poly
C16 = int(os.environ.get("BK_C16", "0"))            # fp16 cell state (ACT_TANH only)
POOL_ADD = int(os.environ.get("BK_POOL_ADD", "0"))  # c'=ig+fc on gpsimd

CLAMP_C = 2.6
# deg-9 odd minimax coeffs for tanh on [0, 2.6] (max abs err 2.7e-3)
TA = (0.9866325884863426, -0.278550831175462, 0.0637625184246867)
TB = (-0.008001787662182125, 0.00040027875656558184)

F32 = mybir.dt.float32
F16 = mybir.dt.float16
AF = mybir.ActivationFunctionType
ALU = mybir.AluOpType

_BK_OPS = None


def _register_custom_ops():
    """Register our fused DVE ops in dve_ops.OPS (idempotent)."""
    global _BK_OPS
    if _BK_OPS is not None:
        return _BK_OPS
    import concourse.dve_ops as DO
    from concourse.dve_spec import (Spec, Src0, Src1, C0, C1, C2, One,
                                    lower, minn, maxx, sq)
    from concourse.dve_uop import DveOpSpec

    have = {op.name: op for op in DO.OPS if op.name.startswith("BK_")}
    if have:
        _BK_OPS = have
        return have

    y = sq(Src0)
    y2 = y * y
    specs = {
        # ig = si * (2*sg - 1)
        "BK_IG_TANH": Spec(
            body=Src0 * (Src1 + Src1 - One),
            reference=lambda in0, in1, s0, s1, imm2: in0 * (2.0 * in1 - 1.0)),
        # c' = clamp(ig + fc, s0, s1)
        "BK_CLAMP_ADD": Spec(
            body=minn(maxx(Src0 + Src1, C0), C1),
            reference=lambda in0, in1, s0, s1, imm2: np.clip(
                in0 + in1, s0, s1)),
        # r = x * (C0 + C1 y + C2 y^2)
        "BK_TANH_A": Spec(
            body=((C0 + C1 * y) + C2 * y2) * Src0,
            reference=lambda in0, s0, s1, imm2: in0 * (
                s0 + s1 * in0 * in0 + imm2 * (in0 * in0) ** 2)),
        # T = r + (x * y^3) * (C0 + C1 y)
        "BK_TANH_B": Spec(
            body=Src1 + (Src0 * (y * y2)) * (C0 + C1 * y),
            reference=lambda in0, in1, s0, s1, imm2: in1 + in0 * (
                in0 * in0) ** 3 * (s0 + s1 * in0 * in0)),
    }
    out = {}
    for name, spec in specs.items():
        row = DO._CUSTOM_DVE_ROW_BASE + len(DO.OPS)
        shas = {}
        for ver in ("v3", "v4"):
            tmp = DveOpSpec(name=name, opcode=row,
                            uops=lower(spec, ver=ver),
                            rd1_en=DO.has_src1(spec))
            shas[ver] = tmp.sha(ver)
        op = DO.DveOp(name, spec, subdim=False, uops_sha=shas)
        DO.OPS.append(op)
        DO.CUSTOM_DVE_SPECS[name] = spec
        DO._SUB_OPCODE_FOR_NAME[name] = row
        out[name] = op
    _BK_OPS = out
    return out


def _gate_perm_scale():
    """Tile order [f|g|i|o] (4 x 128-col tiles per gate); g scaled x2.

    Keras order along 4U: [i(0:U), f(U:2U), g(2U:3U), o(3U:4U)].
    """
    idx = []
    for g0 in (U, 2 * U, 0, 3 * U):  # f, g, i, o
        idx.append(np.arange(g0, g0 + U))
    perm = np.concatenate(idx)
    scale = np.ones(G, np.float32)
    scale[U:2 * U] = 2.0  # g (new position)
    return perm, scale


# z column groups in tile space: f = tiles 0..3, g = 4..7, i = 8..11, o = 12..15
_GRP_OPTS = {
    "f_gi_o": ((0, 4), (4, 12), (12, 16)),
    "f_gio": ((0, 4), (4, 16)),
    "fgi_o": ((0, 12), (12, 16)),
    "f_g_i_o": ((0, 4), (4, 8), (8, 12), (12, 16)),
}
GRP = _GRP_OPTS[os.environ.get("BK_GRP", "f_gi_o")]


def _emit(tc, nc, xT, U16, W16, biasT, ind16, hsT, t_steps):
    b = B_LOC
    ops = _register_custom_ops()
    with ExitStack() as es:
        consts = es.enter_context(tc.tile_pool(name="consts", bufs=1))

        u_t = consts.tile([128, NK, NT, 128], F16, tag="u")
        nc.sync.dma_start(out=u_t, in_=U16)
        w_t = consts.tile([128, NK, NT, 128], F16, tag="w")
        nc.sync.dma_start(out=w_t, in_=W16)
        bias_t = consts.tile([NT, 128], F16, tag="bias")
        nc.sync.dma_start(out=bias_t, in_=biasT)
        ind_t = consts.tile([NT, NT * b], F16, tag="ind")
        nc.sync.dma_start(out=ind_t, in_=ind16)
        hz = consts.tile([128, CB], F16, tag="hz")
        nc.vector.memset(hz[:], 0.0)

        xp = es.enter_context(tc.tile_pool(name="xp", bufs=3))
        zpools = [es.enter_context(tc.tile_pool(name=f"z{gi}", bufs=ZBUFS,
                                                space="PSUM"))
                  for gi in range(len(GRP))]
        scrp = (es.enter_context(tc.tile_pool(name="scr", bufs=1,
                                              space="PSUM"))
                if NDUM else None)
        sp = es.enter_context(tc.tile_pool(name="sig", bufs=3))
        gp = es.enter_context(tc.tile_pool(name="gates", bufs=3))
        cp = es.enter_context(tc.tile_pool(name="c", bufs=2))
        op = es.enter_context(tc.tile_pool(name="out", bufs=3))

        scr = scrp.tile([128, 512], F32, tag="scr") if NDUM else None

        n_xblk = (t_steps + XB - 1) // XB
        x_tiles = {}

        def fetch_x(blk):
            if blk >= n_xblk or blk in x_tiles:
                return
            xt = xp.tile([128, XB, NK, b], F16, tag="x", name=f"x_{blk}")
            w = XB * NK * b
            nc.sync.dma_start(out=xt, in_=xT[:, blk * w:(blk + 1) * w])
            x_tiles[blk] = xt

        fetch_x(0)
        fetch_x(1)

        c_prev = cp.tile([128, CB],
                         F16 if (C16 and ACT_TANH) else F32,
                         tag="c", name="c_init")
        nc.vector.memset(c_prev[:], 0.0)
        h_prev = hz

        def new_ztiles(t):
            return tuple(
                zpools[gi].tile([128, (g1 - g0) * b], F32, tag=f"z{gi}",
                                name=f"z{gi}_{t}")
                for gi, (g0, g1) in enumerate(GRP))

        def emit_wpart(zt, t):
            """bias + W x_t into the three psum tiles (start of accum)."""
            blk, off = divmod(t, XB)
            xt = x_tiles[blk]
            for gi, (g0, g1) in enumerate(GRP):
                z = zt[gi]
                nc.tensor.matmul(z, bias_t, ind_t[:, g0 * b:g1 * b],
                                 start=True, stop=False,
                                 skip_group_check=True)
                for c in range(NK):
                    rhs = xt[:, off, c, :]
                    for g in range(g0, g1):
                        nc.tensor.matmul(z[:, (g - g0) * b:(g - g0 + 1) * b],
                                         w_t[:, c, g, :], rhs,
                                         start=False, stop=False,
                                         skip_group_check=True)

        z_cur = new_ztiles(0)
        emit_wpart(z_cur, 0)

        otile = op.tile([128, OB, CB], F16, tag="o", name="ot_0")

        IG = ops["BK_IG_TANH"]
        CLA = ops["BK_CLAMP_ADD"]
        PTA = ops["BK_TANH_A"]
        PTB = ops["BK_TANH_B"]

        for t in range(t_steps):
            # ---- U-part of step t (waits on h(t-1)), group order f, gi, o --
            for gi, (g0, g1) in enumerate(GRP):
                z = z_cur[gi]
                for c in range(NK):
                    rhs = h_prev[:, c * b:(c + 1) * b]
                    for g in range(g0, g1):
                        nc.tensor.matmul(z[:, (g - g0) * b:(g - g0 + 1) * b],
                                         u_t[:, c, g, :], rhs,
                                         start=False, stop=(c == NK - 1),
                                         skip_group_check=True)

            # ---- sigmoids (ACT), in group order ----
            sig_parts = []
            for gi, (g0, g1) in enumerate(GRP):
                sgt = sp.tile([128, (g1 - g0) * b], F16, tag=f"sig{gi}",
                              name=f"sig{gi}_{t}")
                nc.scalar.activation(sgt, z_cur[gi], AF.Sigmoid)
                sig_parts.append((g0, g1, sgt))

            def gate_slice(t0):
                # gate tiles: f=0..3, g=4..7, i=8..11, o=12..15
                for g0, g1, sgt in sig_parts:
                    if g0 <= t0 < g1:
                        return sgt[:, (t0 - g0) * b:(t0 - g0 + 4) * b]
                raise AssertionError
            sigf = gate_slice(0)
            sg = gate_slice(4)
            si = gate_slice(8)
            sigo = gate_slice(12)

            # ---- cell chain ----
            cdt = F16 if (C16 and ACT_TANH) else F32
            fc = gp.tile([128, CB], cdt, tag="fc", name=f"fc_{t}")
            fc_eng = nc.gpsimd if POOL_FC else nc.vector
            fc_eng.tensor_mul(fc, sigf, c_prev)
            ig = gp.tile([128, CB], cdt, tag="ig", name=f"ig_{t}")
            nc.vector._custom_dve(IG, out=ig, in0=si, in1=sg)
            cn = cp.tile([128, CB], cdt, tag="c", name=f"c_{t}")
            tch = gp.tile([128, CB], F16, tag="tch", name=f"tch_{t}")
            if ACT_TANH:
                add_eng = nc.gpsimd if POOL_ADD else nc.vector
                add_eng.tensor_add(cn, ig, fc)
                nc.scalar.activation(tch, cn, AF.Tanh)
            else:
                nc.vector._custom_dve(CLA, out=cn, in0=ig, in1=fc,
                                      s0=-CLAMP_C, s1=CLAMP_C)
                pr = gp.tile([128, CB], F32, tag="pr", name=f"pr_{t}")
                nc.vector._custom_dve(PTA, out=pr, in0=cn,
                                      s0=TA[0], s1=TA[1], imm2=TA[2])
                nc.vector._custom_dve(PTB, out=tch, in0=cn, in1=pr,
                                      s0=TB[0], s1=TB[1])
            hsl = otile[:, t % OB, :]
            h_eng = nc.gpsimd if POOL_H else nc.vector
            h_eng.tensor_mul(hsl, sigo, tch)

            h_prev = hsl
            c_prev = cn

            # ---- output DMA every OB steps ----
            if t % OB == OB - 1 or t == t_steps - 1:
                t0 = (t // OB) * OB
                nc.sync.dma_start(
                    out=hsT[:, t0 * CB:(t + 1) * CB],
                    in_=otile[:, 0:(t - t0 + 1), :])
                if t != t_steps - 1:
                    otile = op.tile([128, OB, CB], F16, tag="o",
                                    name=f"ot_{t + 1}")

            # ---- x prefetch ----
            if t % XB == 0:
                fetch_x(t // XB + 2)

            # ---- W-part of step t+1 (PE filler, no recurrence dep) ----
            if t + 1 < t_steps:
                z_next = new_ztiles(t + 1)
                emit_wpart(z_next, t + 1)
                z_cur = z_next

            # ---- dummy PE filler to hold p-state ----
            for dmy in range(NDUM):
                nc.tensor.matmul(scr, u_t[:, 0, 0, :],
                                 u_t[:, dmy % NK, 0:4, :],
                                 start=True, stop=True,
                                 skip_group_check=True)


def build_program(t_steps=T):
    _register_custom_ops()
    nc = bacc.Bacc("TRN2", target_bir_lowering=False, debug=False,
                   num_devices=NCORE)
    xT = nc.dram_tensor("xT", [128, t_steps * NK * B_LOC], F16,
                        kind="ExternalInput").ap()
    U16 = nc.dram_tensor("U16", [128, NK, NT, 128], F16,
                         kind="ExternalInput").ap()
    W16 = nc.dram_tensor("W16", [128, NK, NT, 128], F16,
                         kind="ExternalInput").ap()
    biasT = nc.dram_tensor("biasT", [NT, 128], F16, kind="ExternalInput").ap()
    ind16 = nc.dram_tensor("ind16", [NT, NT * B_LOC], F16,
                           kind="ExternalInput").ap()
    hsT = nc.dram_tensor("hsT", [128, t_steps * CB], F16,
                         kind="ExternalOutput").ap()
    with tile.TileContext(nc) as tc:
        _emit(tc, nc, xT, U16, W16, biasT, ind16, hsT, t_steps)
    nc.compile()
    return nc


_CACHE = {}


def _get_program(t_steps=T):
    key = t_steps
    if key not in _CACHE:
        _CACHE[key] = build_program(t_steps)
    return _CACHE[key]


def make_in_maps(xf, xb, Wf, Uf, bf, Wb, Ub, bb, t_steps=T):
    perm, scale = _gate_perm_scale()
    packs = {}
    for d, (W, Urec, bias) in enumerate(((Wf, Uf, bf), (Wb, Ub, bb))):
        Wp = (W[:, perm] * scale).astype(np.float16)
        Up = (Urec[:, perm] * scale).astype(np.float16)
        bp = (bias[perm] * scale).astype(np.float16)
        # [k-chunk, 128, tile, 128] -> lhsT tiles [128(k), NK, NT, 128(m)]
        U16 = np.ascontiguousarray(
            Up.reshape(NK, 128, NT, 128).transpose(1, 0, 2, 3))
        W16 = np.ascontiguousarray(
            Wp.reshape(NK, 128, NT, 128).transpose(1, 0, 2, 3))
        biasT = np.ascontiguousarray(bp.reshape(NT, 128))
        packs[d] = (U16, W16, biasT)
    ind = np.zeros((NT, NT, B_LOC), np.float16)
    for k in range(NT):
        ind[k, k, :] = 1.0
    ind16 = np.ascontiguousarray(ind.reshape(NT, NT * B_LOC))
    in_maps = []
    for core in range(NCORE):
        d, j = divmod(core, NDIR_CORES)
        x = (xf if d == 0 else xb)[B_LOC * j:B_LOC * (j + 1), :t_steps]
        # xT[p, (t, k, b)] = x[b, t, k*128 + p]
        xT = np.ascontiguousarray(
            x.reshape(B_LOC, t_steps, NK, 128)
             .transpose(3, 1, 2, 0)
             .reshape(128, t_steps * NK * B_LOC)).astype(np.float16)
        U16, W16, biasT = packs[d]
        in_maps.append({"xT": xT, "U16": U16, "W16": W16, "biasT": biasT,
                        "ind16": ind16})
    return in_maps


def kernel(xf, xb, Wf, Uf, bf, Wb, Ub, bb):
    xf = np.asarray(xf, np.float32)
    xb = np.asarray(xb, np.float32)
    Wf = np.asarray(Wf, np.float32)
    Uf = np.asarray(Uf, np.float32)
    bf = np.asarray(bf, np.float32)
    Wb = np.asarray(Wb, np.float32)
    Ub = np.asarray(Ub, np.float32)
    bb = np.asarray(bb, np.float32)

    nc = _get_program()
    in_maps = make_in_maps(xf, xb, Wf, Uf, bf, Wb, Ub, bb)
    res = run_bass_kernel_spmd(nc, in_maps, list(range(NCORE)))

    out = np.empty((B, T, 2 * U), np.float32)
    for core in range(NCORE):
        d, j = divmod(core, NDIR_CORES)
        hsv = np.asarray(res.results[core]["hsT"])  # [128, T*CB] f16
        # hsT[p, (t, c, b)] -> out[b, t, d*512 + c*128 + p]
        hs = hsv.reshape(128, T, NK, B_LOC).transpose(3, 1, 2, 0)
        out[B_LOC * j:B_LOC * (j + 1), :, U * d:U * (d + 1)] = \
            hs.reshape(B_LOC, T, U).astype(np.float32)
    return out
